# revision 15
# baseline (speedup 1.0000x reference)
# Trainium2 Bass kernel for nn_Block_88201448390974 (dense transformer block).
#
# Sharding: pure data-parallel over batch B=16 across 8 NeuronCores
# (2 batches per core, zero collectives).
#
# Per-core dataflow (fp8e4 matmuls with DoubleRow double-pumping on PE,
# fp32 PSUM accumulation; K=384 contractions run as one DoubleRow pass
# (K=256) plus one plain fp8 pass (K=128)):
#   LN1 (token-partition layout, bn_stats; rstd = exp(-0.5*ln(var+eps)) so the
#   ScalarE table set stays in natural_log_exp through the attention phase)
#   PE-transpose h -> hT fp8 [c, tok] (dtype conversion rides the PSUM->SBUF copy)
#   qT,kT transposed-out bf16 [f, tok]; v natural fp8 [tok, f] with a ones column
#   scores^T[j,i] = kT.T @ qT  bf16  (j on partitions; no max-subtraction: the
#   logits are O(1) for this problem, exp is safe in fp32)
#   exp on ScalarE (scale = att_scale / w-scales) -> e fp8, per-head [j, i] tiles
#   U[i, (d|Z)] = e.T @ [v|1]  DoubleRow over j-tile pairs, natural orientation:
#   the softmax denominator Z lands per-partition, so normalize is a cheap
#   per-partition tensor_scalar on DVE (no reciprocal-broadcast dance)
#   o -> oT fp8 via PE transpose; proj natural-out + residual (the fp8 weight
#   upscaling is compensated in the scalar_tensor_tensor residual add);
#   LN2; fc1 + exact Gelu (compensation via activation scale) -> m fp8;
#   fc2 DoubleRow over hidden pairs + residual.
#
# The two batches are software-pipelined at emission time (engines execute
# their streams near emission order): attention(b) hides LN/QKV(b+1) and
# MLP(b) hides attention(b+1). ScalarE (exp+gelu) is the roofline engine;
# PSUM-reading vector work sits on DVE, SBUF-only work on gpsimd.
#
# fp8 scale folds (weights are upscaled 8x host-side so w-sigma ~0.16 clears
# the e4m3 subnormal cliff at 2^-6): exp scale 1/(8*8) * att_scale, gelu
# scale 1/8, proj residual comp 1/64 (v-scale * proj-scale), fc2 comp 1/8.

import numpy as np
import ml_dtypes

import concourse.bass as bass
import concourse.bacc as bacc
import concourse.mybir as mybir
import concourse.tile as tile
from concourse.bass_utils import run_bass_kernel_spmd
from concourse.masks import make_identity

FP32 = mybir.dt.float32
BF16 = mybir.dt.bfloat16
FP8 = mybir.dt.float8e4
AF = mybir.ActivationFunctionType
ALU = mybir.AluOpType
DR = mybir.MatmulPerfMode.DoubleRow

B, N, C, H = 16, 1024, 384, 6
Dh = C // H          # 64
Dff = 4 * C          # 1536
NCORES = 8
BL = B // NCORES     # batches per core
P = 128
TPB = N // P         # 8 token tiles per batch
CC = C // P          # 3 feature chunks of 128
FCH = Dff // P       # 12 hidden chunks of 128
NHALF = N // 512     # 2 moving-dim halves of 512
LN_EPS = 1e-5
ATT_SCALE = Dh ** -0.5

S_QKV = 8.0          # host-side upscale of qkv weights (fp8 subnormal dodge)
S_PROJ = 8.0
S_FC1 = 8.0
S_FC2 = 8.0
EXP_SCALE = ATT_SCALE / (S_QKV * S_QKV)
GELU_SCALE = 1.0 / S_FC1
PROJ_COMP = 1.0 / (S_QKV * S_PROJ)   # v-scale * proj-scale
FC2_COMP = 1.0 / S_FC2


def _interleave(*gens):
    gens = [g for g in gens if g is not None]
    while gens:
        for g in list(gens):
            try:
                next(g)
            except StopIteration:
                gens.remove(g)


def _chain(*gens):
    for g in gens:
        yield from g


def build_nc(debug=False, repeat=1):
    nc = bacc.Bacc()
    x_d = nc.declare_dram_parameter("x", [BL, N, C], FP32, isOutput=False)
    qkvw_d = nc.declare_dram_parameter("qkv_wT", [C, 3 * C], FP8, isOutput=False)
    projw_d = nc.declare_dram_parameter("proj_wT", [C, C], FP8, isOutput=False)
    fc1w_d = nc.declare_dram_parameter("fc1_wT", [C, Dff], FP8, isOutput=False)
    fc1b_d = nc.declare_dram_parameter("fc1_b", [Dff], FP32, isOutput=False)
    fc2w_d = nc.declare_dram_parameter("fc2_wT", [Dff, C], FP8, isOutput=False)
    out_d = nc.declare_dram_parameter("out", [BL, N, C], FP32, isOutput=True)

    with tile.TileContext(nc) as tc:
        with (
            tc.tile_pool(name="consts", bufs=1) as consts,
            tc.tile_pool(name="weights", bufs=1) as weights,
            tc.tile_pool(name="acts", bufs=1) as acts,
            tc.tile_pool(name="lnst", bufs=2) as lnst,
            tc.tile_pool(name="psum", bufs=1, space="PSUM") as psum,
        ):
            from concourse.hw_specs import get_activation_tables
            _set_names = list(get_activation_tables(nc.m.arch).keys())
            NLX_SET = _set_names.index("natural_log_exp_and_others")

            def load_nlx_set(after=None):
                inst = nc.scalar.add_instruction(mybir.InstLoadActFuncSet(
                    name=nc.get_next_instruction_name(), ins=[], outs=[],
                    act_func_set_id=NLX_SET))
                if after is not None:
                    bass._add_dep_helper(inst.ins, after.ins, sync=False,
                                         reason="pin table load after gelu phase")
                return inst

            ident = consts.tile([P, P], BF16, tag="ident")
            make_identity(nc, ident)
            eps_tile = consts.tile([P, 1], FP32, tag="eps")
            nc.vector.memset(eps_tile, LN_EPS)

            # --- weights to SBUF (emitted after the first x tiles so the
            # startup HBM bandwidth goes to x first; gpsimd queues) ---
            qkvw_sb = weights.tile([P, CC, 3 * C], FP8, tag="qkvw")
            projw_sb = weights.tile([P, CC, C], FP8, tag="projw")
            fc1w_sb = weights.tile([P, CC, Dff], FP8, tag="fc1w")
            fc1b_sb = weights.tile([P, FCH], FP32, tag="fc1b")
            fc2w_sb = weights.tile([P, FCH, C], FP8, tag="fc2w")

            def load_weights_early():
                nc.gpsimd.dma_start(out=qkvw_sb, in_=qkvw_d.rearrange("(cc p) f -> p cc f", p=P))

            def load_weights_late(after=None):
                for w_sb, w_d, pat in [
                    (projw_sb, projw_d, "(cc p) f -> p cc f"),
                    (fc1w_sb, fc1w_d, "(cc p) f -> p cc f"),
                    (fc1b_sb, fc1b_d, "(fc p) -> p fc"),
                    (fc2w_sb, fc2w_d, "(fc p) c -> p fc c"),
                ]:
                    d = nc.gpsimd.dma_start(out=w_sb, in_=w_d.rearrange(pat, p=P))
                    if after is not None:
                        bass._add_dep_helper(d.ins, after.ins, sync=True,
                                             reason="defer weight load past x")

            st = {}   # per-batch-slot live tiles

            def layernorm_batch(x_sb, tag):
                """rstd = exp(-0.5*ln(var+eps)); done in two half-batches so
                downstream transposes can start after 4 tiles, not 8."""
                mv8 = lnst.tile([P, TPB, 2], FP32, tag=f"mv8_{tag}", bufs=2)
                rstd8 = lnst.tile([P, TPB], FP32, tag=f"rstd_{tag}", bufs=2)
                HB = TPB // 2
                for hb in range(2):
                    for it in range(hb * HB, (hb + 1) * HB):
                        stats = lnst.tile([P, nc.vector.BN_STATS_DIM], FP32,
                                          tag=f"st_{tag}", bufs=3)
                        nc.vector.bn_stats(out=stats, in_=x_sb[:, it, :])
                        nc.vector.bn_aggr(out=mv8[:, it, :], in_=stats)
                    lnv = lnst.tile([P, HB], FP32, tag=f"lnv_{tag}", bufs=2)
                    nc.scalar.activation(out=lnv, in_=mv8[:, hb * HB:(hb + 1) * HB, 1],
                                         func=AF.Ln, bias=eps_tile[:, 0:1])
                    nc.scalar.activation(out=rstd8[:, hb * HB:(hb + 1) * HB],
                                         in_=lnv, func=AF.Exp, scale=-0.5)
                return mv8, rstd8

            def transpose_128x384(src_ap, dst_sb, it):
                """3x PE transpose of src [P, 384] into dst [P, CC, it-slice],
                staged through a bf16 view of a big psum buffer."""
                big = psum.tile([P, N], FP32, tag="big", bufs=2)
                tp = big.bitcast(BF16)[:, 0:C].rearrange("p (c q) -> p c q", c=CC)
                for cc in range(CC):
                    nc.tensor.transpose(tp[:, cc, :], src_ap[:, cc * P:(cc + 1) * P], ident)
                nc.vector.tensor_copy(
                    out=dst_sb[:, :, it * P:(it + 1) * P], in_=tp)

            def normalize_transpose(x_sb, mv8, rstd8, dst_sb, it):
                """(x-m)*rstd -> bf16, PE transpose, copy to dst (dtype of dst)."""
                h_bf = acts.tile([P, C], BF16, tag="h_bf", bufs=3)
                nc.gpsimd.tensor_scalar(
                    out=h_bf, in0=x_sb[:, it, :],
                    scalar1=mv8[:, it, 0:1], scalar2=rstd8[:, it:it + 1],
                    op0=ALU.subtract, op1=ALU.mult)
                transpose_128x384(h_bf, dst_sb, it)

            def mm_k384(ps_out, lhs_sb, lhs_lo, lhs_hi, rhs_sb, rhs_lo, rhs_hi):
                """K=384 contraction: DoubleRow (chunks 0,1) + plain (chunk 2).
                lhs/rhs indexed [P, CC, free]; slices are free-dim ranges."""
                nc.tensor.matmul(
                    ps_out,
                    lhsT=lhs_sb[:, 0:2, lhs_lo:lhs_hi],
                    rhs=rhs_sb[:, 0:2, rhs_lo:rhs_hi],
                    start=True, stop=False, perf_mode=DR)
                nc.tensor.matmul(
                    ps_out,
                    lhsT=lhs_sb[:, 2, lhs_lo:lhs_hi],
                    rhs=rhs_sb[:, 2, rhs_lo:rhs_hi],
                    start=False, stop=True)

            def stage_a(b):
                """x load + LN1 + transpose + qkv."""
                if b % 2 == 0:
                    load_nlx_set()   # ln+exp resident through LN1+attention
                x_sb = acts.tile([P, TPB, C], FP32, tag="x", bufs=2)
                st[b] = {"x": x_sb}
                for it in range(TPB):
                    nc.sync.dma_start(out=x_sb[:, it, :],
                                      in_=x_d[b % BL, it * P:(it + 1) * P, :])
                if b == 0:
                    load_weights_early()
                yield
                mv8, rstd8 = layernorm_batch(x_sb, "ln1")
                yield
                hT_sb = acts.tile([P, CC, N], FP8, tag="hT", bufs=2)
                st[b]["hT"] = hT_sb
                for it in range(TPB):
                    normalize_transpose(x_sb, mv8, rstd8, hT_sb, it)
                    yield
                qkT_sb = acts.tile([P, 6, N], BF16, tag="qkT", bufs=2)
                st[b]["qkT"] = qkT_sb
                # emit q0,k0,q1,k1,q2,k2 so head 0's scores unblock after
                # two chunks, shrinking the fill before the first exp
                for fch in (0, 3, 1, 4, 2, 5):
                    ps = psum.tile([P, N], FP32, tag="big", bufs=2)
                    for half in range(NHALF):
                        mm_k384(ps[:, half * 512:(half + 1) * 512],
                                qkvw_sb, fch * P, (fch + 1) * P,
                                hT_sb, half * 512, (half + 1) * 512)
                    cp = nc.vector.tensor_copy(out=qkT_sb[:, fch, :], in_=ps)
                    if fch == 0:
                        st[b]["x_anchor"] = cp
                    yield
                v_sb = acts.tile([P, TPB, H, Dh + 1], FP8, tag="v", bufs=2)
                st[b]["v"] = v_sb
                nc.gpsimd.memset(v_sb[:, :, :, Dh:Dh + 1], 1.0)
                for jt in range(TPB):
                    psv = psum.tile([P, N], FP32, tag="big", bufs=2)
                    mm_k384(psv[:, 0:C],
                            hT_sb, jt * P, (jt + 1) * P,
                            qkvw_sb, 2 * C, 3 * C)
                    nc.vector.tensor_copy(
                        out=v_sb[:, jt, :, 0:Dh],
                        in_=psv[:, 0:C].rearrange("p (h d) -> p h d", h=H))
                    yield

            def stage_b(b):
                """attention, one head at a time; scores bf16, exp -> fp8,
                U = e.T @ [v|1] natural orientation, DoubleRow over jt pairs."""
                qkT_sb, v_sb = st[b]["qkT"], st[b]["v"]
                if b % 2 == 0:
                    # during attention the DMA path is idle; anchor past x load
                    load_weights_late(after=st[b].get("x_anchor"))
                o_sb = acts.tile([P, TPB, C], BF16, tag="o", bufs=2)
                st[b]["o"] = o_sb

                def emit_u(job):
                    """one DR accumulation pass (j-tile pair) for all i-tiles;
                    after the stop pass, normalize that head's o rows."""
                    e_all, u_ps, h, t = job
                    for it in range(TPB):
                        nc.tensor.matmul(
                            u_ps[it // 4][:, it % 4, :],
                            lhsT=e_all[:, 2 * t:2 * t + 2, it * P:(it + 1) * P],
                            rhs=v_sb[:, 2 * t:2 * t + 2, h, :],
                            start=(t == 0), stop=(t == TPB // 2 - 1),
                            perf_mode=DR)
                    if t == TPB // 2 - 1:
                        for g in range(2):
                            zr = acts.tile([P, 4], FP32, tag="zr", bufs=4)
                            nc.vector.reciprocal(zr, u_ps[g][:, :, Dh])
                            for q in range(4):
                                it = g * 4 + q
                                nc.vector.tensor_scalar(
                                    out=o_sb[:, it, h * Dh:(h + 1) * Dh],
                                    in0=u_ps[g][:, q, 0:Dh],
                                    scalar1=zr[:, q:q + 1], scalar2=None,
                                    op0=ALU.mult)

                # U passes are emitted one j-tile late so the next scores
                # matmul always sits ahead of the exp-waiting U block in the
                # in-order PE stream (otherwise ScalarE starves every other jt)
                pending = []
                for h in range(H):
                    po = (h % 2) * Dh
                    qc, kc = h // 2, 3 + h // 2
                    e_all = acts.tile([P, TPB, N], FP8, tag="e", bufs=2)
                    u_t0 = psum.tile([P, C], FP32, tag="cps", bufs=4)
                    u_t1 = psum.tile([P, C], FP32, tag="cps", bufs=4)
                    u_ps = [
                        u_t0[:, 0:4 * (Dh + 1)].rearrange("p (q f) -> p q f", f=Dh + 1),
                        u_t1[:, 0:4 * (Dh + 1)].rearrange("p (q f) -> p q f", f=Dh + 1),
                    ]
                    for jt in range(TPB):
                        ps_s = psum.tile([P, N], FP32, tag="big", bufs=2)
                        for half in range(NHALF):
                            nc.tensor.matmul(
                                ps_s[:, half * 512:(half + 1) * 512],
                                lhsT=qkT_sb[po:po + Dh, kc, jt * P:(jt + 1) * P],
                                rhs=qkT_sb[po:po + Dh, qc, half * 512:(half + 1) * 512],
                                start=True, stop=True)
                        nc.scalar.activation(out=e_all[:, jt, :], in_=ps_s,
                                             func=AF.Exp, scale=EXP_SCALE)
                        if pending:
                            emit_u(pending.pop(0))
                        if jt % 2 == 1:
                            pending.append((e_all, u_ps, h, (jt - 1) // 2))
                        yield
                for job in pending:
                    emit_u(job)
                    yield

            def stage_c1a(b):
                """o transpose + proj + residual (no ScalarE work)."""
                x_sb, o_sb = st[b]["x"], st[b]["o"]
                oT_sb = acts.tile([P, CC, N], FP8, tag="oT", bufs=2)
                for it in range(TPB):
                    transpose_128x384(o_sb[:, it, :], oT_sb, it)
                    yield
                x2_sb = acts.tile([P, TPB, C], FP32, tag="x2", bufs=2)
                st[b]["x2"] = x2_sb
                for it in range(TPB):
                    ps = psum.tile([P, C], FP32, tag="cps", bufs=4)
                    mm_k384(ps,
                            oT_sb, it * P, (it + 1) * P,
                            projw_sb, 0, C)
                    nc.vector.scalar_tensor_tensor(
                        out=x2_sb[:, it, :], in0=ps, scalar=PROJ_COMP,
                        in1=x_sb[:, it, :], op0=ALU.mult, op1=ALU.add)
                    yield

            def stage_c1b(b):
                """LN2: stats + rstd (nlx table) + normalize + transposes."""
                if b % 2 == 1 and "last_gelu" in st.get(b - 1, {}):
                    # re-seed ln+exp set, pinned after the b-1 gelu phase
                    load_nlx_set(after=st[b - 1]["last_gelu"])
                x2_sb = st[b]["x2"]
                mv8b, rstd8b = layernorm_batch(x2_sb, "ln2")
                yield
                h2T_sb = acts.tile([P, CC, N], FP8, tag="h2T", bufs=2)
                st[b]["h2T"] = h2T_sb
                for it in range(TPB):
                    normalize_transpose(x2_sb, mv8b, rstd8b, h2T_sb, it)
                    yield

            def stage_c2a(b):
                """fc1 + gelu -> m fp8 (ScalarE gelu table)."""
                h2T_sb = st[b]["h2T"]
                m_sb = acts.tile([P, FCH, N], FP8, tag="m", bufs=1)
                st[b]["m"] = m_sb
                for fch in range(FCH):
                    ps = psum.tile([P, N], FP32, tag="big", bufs=2)
                    for half in range(NHALF):
                        mm_k384(ps[:, half * 512:(half + 1) * 512],
                                fc1w_sb, fch * P, (fch + 1) * P,
                                h2T_sb, half * 512, (half + 1) * 512)
                    g = nc.scalar.activation(
                        out=m_sb[:, fch, :], in_=ps,
                        func=AF.Gelu, bias=fc1b_sb[:, fch:fch + 1],
                        scale=GELU_SCALE)
                    st[b]["last_gelu"] = g
                    yield

            def stage_c2b(b):
                """fc2 (DoubleRow over hidden pairs) + residual + store."""
                x2_sb, m_sb = st[b]["x2"], st[b]["m"]
                for it in range(TPB):
                    ps = psum.tile([P, C], FP32, tag="cps", bufs=4)
                    for t in range(FCH // 2):
                        nc.tensor.matmul(
                            ps,
                            lhsT=m_sb[:, 2 * t:2 * t + 2, it * P:(it + 1) * P],
                            rhs=fc2w_sb[:, 2 * t:2 * t + 2, :],
                            start=(t == 0), stop=(t == FCH // 2 - 1),
                            perf_mode=DR)
                    y_sb = acts.tile([P, C], FP32, tag="y", bufs=3)
                    nc.vector.scalar_tensor_tensor(
                        out=y_sb, in0=ps, scalar=FC2_COMP,
                        in1=x2_sb[:, it, :], op0=ALU.mult, op1=ALU.add)
                    nc.sync.dma_start(
                        out=out_d[b % BL, it * P:(it + 1) * P, :], in_=y_sb)
                    yield

            # software pipeline (per repeat pair) with true interleaved
            # emission — engines execute their streams in-order, so cross-
            # phase overlap requires interleaving at emission time:
            #   [A0 || C2b_1'(prev)]; [B0 || A1]; [B1 || C1a_0];
            #   [C1b_0;C2a_0;C1b_1 || C1a_1]; [C2b_0 || C2a_1]; (C2b_1 -> next)
            # ScalarE stream: rstd1(b0), exps(b0) | rstd1(b1), exps(b1) |
            # rstd2(b0), gelus(b0), rstd2(b1), gelus(b1) — table swaps only
            # at the exp->gelu and gelu->ln boundaries.
            prev_tail = None
            for rep in range(repeat):
                b0, b1 = 2 * rep, 2 * rep + 1
                _interleave(stage_a(b0), prev_tail)
                _interleave(stage_b(b0), stage_a(b1))
                _interleave(stage_b(b1), stage_c1a(b0))
                _interleave(_chain(stage_c1b(b0), stage_c2a(b0), stage_c1b(b1)),
                            stage_c1a(b1))
                _interleave(stage_c2b(b0), stage_c2a(b1))
                prev_tail = stage_c2b(b1)
            _interleave(prev_tail)
    return nc


_NC_CACHE = None


def _get_nc():
    global _NC_CACHE
    if _NC_CACHE is None:
        nc = build_nc()
        nc.finalize()   # runs Bacc passes (reg alloc, sync-wait splitting)
        _NC_CACHE = nc
    return _NC_CACHE


def _prep_in_maps(inputs):
    f32 = lambda a: np.asarray(a, dtype=np.float32)
    e4 = lambda a: np.ascontiguousarray(a.astype(ml_dtypes.float8_e4m3))
    x = f32(inputs["x"])
    ln1_g, ln2_g = f32(inputs["ln1_g"]), f32(inputs["ln2_g"])
    gate_h, gate_mlp = f32(inputs["gate_h"]), f32(inputs["gate_mlp"])

    qkv_wT = f32(inputs["qkv_w"]).T.copy()          # [C, 3C]
    qkv_wT *= ln1_g[:, None] * S_QKV                # fold LN1 gain + fp8 upscale
    proj_wT = f32(inputs["proj_w"]).T.copy()        # [C, C]
    proj_wT *= np.repeat(gate_h, Dh)[:, None] * S_PROJ
    fc1_wT = f32(inputs["fc1_w"]).T.copy()          # [C, Dff]
    fc1_wT *= ln2_g[:, None] * S_FC1
    fc2_wT = f32(inputs["fc2_w"]).T.copy()          # [Dff, C]
    fc2_wT *= gate_mlp[:, None] * S_FC2

    shared = {
        "qkv_wT": e4(qkv_wT),
        "proj_wT": e4(proj_wT),
        "fc1_wT": e4(fc1_wT),
        "fc1_b": f32(inputs["fc1_b"]).copy(),
        "fc2_wT": e4(fc2_wT),
    }
    return [dict(shared, x=np.ascontiguousarray(x[c * BL:(c + 1) * BL]))
            for c in range(NCORES)]


def _run(inputs, **kw):
    nc = _get_nc()
    in_maps = _prep_in_maps(inputs)
    return run_bass_kernel_spmd(nc, in_maps, list(range(NCORES)), **kw)


def kernel(**inputs) -> np.ndarray:
    res = _run(inputs)
    return np.concatenate(
        [np.asarray(res.results[i]["out"], dtype=np.float32) for i in range(NCORES)],
        axis=0)
